# revision 2
# baseline (speedup 1.0000x reference)
"""Causal multi-head attention with RoPE on 8 Trainium2 NeuronCores.

Problem: B=2, S=2048, D=1024, H=16 heads, head_dim=64, fp32 in/out.

Sharding (hardcoded): 8 cores = 2 (batch) x 4 (head groups of 4 heads).
Core c handles batch b = c // 4 and heads [hg*4, hg*4+4), hg = c % 4.
Each core computes its 4 heads' attention plus the partial output
projection o_part = attn_part @ wo[:, cols].T; the host sums the 4
partials per batch (the row-parallel reduction) to form the output.

Device dataflow per core (all matmuls in bf16, fp32 accumulation):
  qT/kT projections in transposed layout (channels on partitions),
  RoPE applied in that layout: channels of wq/wk are pre-permuted on
  host so each head's dims are [evens, odds]; the pair-rotation then
  needs the half-swapped vector, obtained with a 128x128 permutation
  matmul, and two elementwise multiplies against cos/sin tables.
  Scores are computed transposed, sT = k_rot @ q_rot.T (Sk on
  partitions), exp applied on ScalarE (scale=1/8 folded in), causal
  masking via memset + one triangular-mask multiply on the diagonal
  128-block. A@V uses exp(sT) blocks as the moving operand with
  stationary [v_h | ones] (M=65), so partition 64 of the accumulator
  carries the softmax denominators. Normalization broadcasts 1/r
  across partitions with an accumulating ones-matmul (hi+lo bf16
  split, so the broadcast is fp32-accurate), then the wo projection
  contracts the 256 channels and streams fp32 results to DRAM.
"""

import numpy as np
import ml_dtypes

import concourse.bass as bass
import concourse.mybir as mybir
import concourse.tile as tile_mod
from concourse.bass_utils import run_bass_kernel_spmd

BF16 = ml_dtypes.bfloat16
dt = mybir.dt

B = 2
S = 2048
D = 1024
H = 16
HD = 64          # head dim
HPC = 4          # heads per core
NCH = HPC * HD   # 256 channels per core
KT = D // 128    # 8 contraction tiles over D
NM = S // 128    # 16 seq tiles of 128
NJ = S // 512    # 4 seq chunks of 512
THETA = 10000.0

_CACHE = {}


def _split_multi_waits(nc):
    # This container's walrus build rejects >1 sync wait per instruction.
    # Hoist extra waits onto InstEventSemaphore carriers placed before the
    # instruction in the same engine's stream.
    for bb in nc.main_func.blocks:
        new_list = []
        for ins in bb.instructions:
            si = getattr(ins, "sync_info", None)
            if si is not None and si.on_wait and len(si.on_wait) > 1:
                waits = list(si.on_wait)
                si.on_wait = [waits[-1]]
                for w in waits[:-1]:
                    ev = mybir.InstEventSemaphore(
                        name=nc.get_next_instruction_name(),
                        engine=ins.engine,
                        ins=[],
                        outs=[],
                        sync_info=mybir.SyncInfo(on_wait=[w], on_update=[]),
                    )
                    nc.register_instruction(ev, overwrite=True)
                    new_list.append(ev)
            new_list.append(ins)
        bb.instructions[:] = new_list


def _build_nc():
    nc = bass.Bass("TRN2", target_bir_lowering=False)

    # Inputs are shipped in SBUF layout (128 partitions first).
    xT = nc.dram_tensor("xT", [128, KT * S], dt.bfloat16, kind="ExternalInput")
    wq = nc.dram_tensor("wq", [128, KT * NCH], dt.bfloat16, kind="ExternalInput")
    wk = nc.dram_tensor("wk", [128, KT * NCH], dt.bfloat16, kind="ExternalInput")
    wv = nc.dram_tensor("wv", [128, KT * NCH], dt.bfloat16, kind="ExternalInput")
    wo = nc.dram_tensor("wo", [128, 2 * D], dt.bfloat16, kind="ExternalInput")
    cosd = nc.dram_tensor("cosd", [128, S], dt.float32, kind="ExternalInput")
    sind = nc.dram_tensor("sind", [128, S], dt.float32, kind="ExternalInput")
    perm = nc.dram_tensor("perm", [128, 128], dt.bfloat16, kind="ExternalInput")
    tri = nc.dram_tensor("tri", [128, 128], dt.bfloat16, kind="ExternalInput")
    out = nc.dram_tensor("o", [S, D], dt.float32, kind="ExternalOutput")

    EXP = mybir.ActivationFunctionType.Exp

    with tile_mod.TileContext(nc) as tc:
        with (
            tc.tile_pool(name="io", bufs=1) as io,
            tc.tile_pool(name="wk1", bufs=3) as wkp,
            tc.tile_pool(name="ep", bufs=4) as ep,
            tc.tile_pool(name="sm", bufs=2) as sm,
            tc.tile_pool(name="ob", bufs=3) as ob,
            tc.tile_pool(name="ps", bufs=3, space="PSUM") as ps,
            tc.tile_pool(name="po", bufs=2, space="PSUM") as po_p,
            tc.tile_pool(name="pb", bufs=2, space="PSUM") as pb_p,
        ):
            xT_sb = io.tile([128, KT * S], dt.bfloat16, tag="xT")
            nc.sync.dma_start(xT_sb[:], xT[:])
            wq_sb = io.tile([128, KT * NCH], dt.bfloat16, tag="wq")
            nc.sync.dma_start(wq_sb[:], wq[:])
            wk_sb = io.tile([128, KT * NCH], dt.bfloat16, tag="wk")
            nc.sync.dma_start(wk_sb[:], wk[:])
            wv_sb = io.tile([128, KT * NCH], dt.bfloat16, tag="wv")
            nc.sync.dma_start(wv_sb[:], wv[:])
            wo_sb = io.tile([128, 2 * D], dt.bfloat16, tag="wo")
            nc.sync.dma_start(wo_sb[:], wo[:])
            cos_sb = io.tile([128, S], dt.float32, tag="cos")
            nc.sync.dma_start(cos_sb[:], cosd[:])
            sin_sb = io.tile([128, S], dt.float32, tag="sin")
            nc.sync.dma_start(sin_sb[:], sind[:])
            perm_sb = io.tile([128, 128], dt.bfloat16, tag="perm")
            nc.sync.dma_start(perm_sb[:], perm[:])
            tri_sb = io.tile([128, 128], dt.bfloat16, tag="tri")
            nc.sync.dma_start(tri_sb[:], tri[:])

            ones_sb = io.tile([1, 64], dt.bfloat16, tag="ones")
            nc.vector.memset(ones_sb[:], 1.0)

            q_sb = io.tile([128, 2 * S], dt.bfloat16, tag="q")
            k_sb = io.tile([128, 2 * S], dt.bfloat16, tag="k")
            v_sb = io.tile([128, NM * HPC * 65], dt.bfloat16, tag="v")
            attn_sb = io.tile([128, 2 * S], dt.bfloat16, tag="attn")

            # ---- qT/kT projections + RoPE (transposed layout) ----
            for dst_sb, w_sb in ((q_sb, wq_sb), (k_sb, wk_sb)):
                for g in range(2):
                    for j in range(NJ):
                        pp = ps.tile([128, 512], dt.float32, tag="ps")
                        for k in range(KT):
                            nc.tensor.matmul(
                                pp[:],
                                w_sb[:, k * NCH + g * 128 : k * NCH + (g + 1) * 128],
                                xT_sb[:, k * S + j * 512 : k * S + (j + 1) * 512],
                                start=(k == 0),
                                stop=(k == KT - 1),
                            )
                        raw = wkp.tile([128, 512], dt.bfloat16, tag="raw")
                        nc.vector.tensor_copy(raw[:], pp[:])
                        pq = ps.tile([128, 512], dt.float32, tag="ps")
                        nc.tensor.matmul(
                            pq[:], perm_sb[:], raw[:], start=True, stop=True
                        )
                        t1 = wkp.tile([128, 512], dt.float32, tag="t1")
                        nc.vector.tensor_mul(
                            t1[:], raw[:], cos_sb[:, j * 512 : (j + 1) * 512]
                        )
                        t2 = wkp.tile([128, 512], dt.float32, tag="t2")
                        nc.vector.tensor_mul(
                            t2[:], pq[:], sin_sb[:, j * 512 : (j + 1) * 512]
                        )
                        nc.vector.tensor_add(
                            dst_sb[:, g * S + j * 512 : g * S + (j + 1) * 512],
                            t1[:],
                            t2[:],
                        )

            # ---- v projection into [v_h | 1] blocks (natural layout) ----
            for m in range(NM):
                pv = ps.tile([128, NCH], dt.float32, tag="ps")
                for k in range(KT):
                    nc.tensor.matmul(
                        pv[:],
                        xT_sb[:, k * S + m * 128 : k * S + (m + 1) * 128],
                        wv_sb[:, k * NCH : (k + 1) * NCH],
                        start=(k == 0),
                        stop=(k == KT - 1),
                    )
                blk = v_sb[:, m * HPC * 65 : (m + 1) * HPC * 65].rearrange(
                    "p (h c) -> p h c", c=65
                )
                nc.vector.tensor_copy(
                    blk[:, :, 0:64], pv[:].rearrange("p (h c) -> p h c", c=64)
                )
                nc.vector.memset(blk[:, :, 64:65], 1.0)

            # ---- attention: sT = k_rot @ q_rot.T, exp, A@V with denominators ----
            for h in range(HPC):
                g = h // 2
                off = 64 * (h % 2)
                for j in range(NJ):
                    nblk = 4 * j + 4
                    po = po_p.tile([65, 512], dt.float32, tag="po")
                    for i in range(nblk):
                        pscr = ps.tile([128, 512], dt.float32, tag="ps")
                        nc.tensor.matmul(
                            pscr[:],
                            k_sb[off : off + 64, g * S + i * 128 : g * S + (i + 1) * 128],
                            q_sb[off : off + 64, g * S + j * 512 : g * S + (j + 1) * 512],
                            start=True,
                            stop=True,
                        )
                        e = ep.tile([128, 512], dt.bfloat16, tag="e")
                        r = i - 4 * j
                        if r < 0:
                            nc.scalar.activation(e[:], pscr[:], EXP, scale=0.125)
                        else:
                            if r > 0:
                                nc.vector.memset(e[:, 0 : 128 * r], 0.0)
                            nc.scalar.activation(
                                e[:, 128 * r : 512],
                                pscr[:, 128 * r : 512],
                                EXP,
                                scale=0.125,
                            )
                            nc.vector.tensor_mul(
                                e[:, 128 * r : 128 * r + 128],
                                e[:, 128 * r : 128 * r + 128],
                                tri_sb[:],
                            )
                        nc.tensor.matmul(
                            po[:],
                            v_sb[:, i * HPC * 65 + h * 65 : i * HPC * 65 + (h + 1) * 65],
                            e[:],
                            start=(i == 0),
                            stop=(i == nblk - 1),
                        )
                    # normalize: out_h = po[0:64] * broadcast(1 / po[64])
                    rec = sm.tile([1, 512], dt.float32, tag="rec")
                    nc.vector.reciprocal(rec[:], po[64:65, :])
                    rhi = sm.tile([1, 512], dt.bfloat16, tag="rhi")
                    nc.vector.tensor_copy(rhi[:], rec[:])
                    rlo = sm.tile([1, 512], dt.bfloat16, tag="rlo")
                    nc.vector.tensor_sub(rlo[:], rec[:], rhi[:])
                    pb = pb_p.tile([64, 512], dt.float32, tag="pb")
                    nc.tensor.matmul(pb[:], ones_sb[:], rhi[:], start=True, stop=False)
                    nc.tensor.matmul(pb[:], ones_sb[:], rlo[:], start=False, stop=True)
                    u_sb = sm.tile([64, 512], dt.float32, tag="u")
                    nc.vector.tensor_copy(u_sb[:], po[0:64, :])
                    nc.vector.tensor_mul(
                        attn_sb[off : off + 64, g * S + j * 512 : g * S + (j + 1) * 512],
                        u_sb[:],
                        pb[:],
                    )

            # ---- output projection: o = attn @ wo_s.T (partial, fp32) ----
            for m in range(NM):
                for n in range(2):
                    pf = ps.tile([128, 512], dt.float32, tag="ps")
                    for g in range(2):
                        nc.tensor.matmul(
                            pf[:],
                            attn_sb[:, g * S + m * 128 : g * S + (m + 1) * 128],
                            wo_sb[:, g * D + n * 512 : g * D + (n + 1) * 512],
                            start=(g == 0),
                            stop=(g == 1),
                        )
                    osb = ob.tile([128, 512], dt.float32, tag="osb")
                    nc.vector.tensor_copy(osb[:], pf[:])
                    nc.sync.dma_start(
                        out[m * 128 : (m + 1) * 128, n * 512 : (n + 1) * 512], osb[:]
                    )

    _split_multi_waits(nc)
    return nc


def _sbuf_layout(a128xN):
    # (T*128, N) -> (128, T*N) with tile t at columns [t*N, (t+1)*N)
    t = a128xN.shape[0] // 128
    n = a128xN.shape[1]
    return np.ascontiguousarray(
        a128xN.reshape(t, 128, n).transpose(1, 0, 2).reshape(128, t * n)
    )


def _host_prep(x, wq, wk, wv, wo, token_positions):
    x = np.asarray(x, dtype=np.float32)
    wq = np.asarray(wq, dtype=np.float32)
    wk = np.asarray(wk, dtype=np.float32)
    wv = np.asarray(wv, dtype=np.float32)
    wo = np.asarray(wo, dtype=np.float32)
    pos = np.asarray(token_positions).astype(np.float32)

    # deinterleave channel order within each head for q/k: [evens, odds]
    de = np.concatenate([np.arange(0, HD, 2), np.arange(1, HD, 2)])

    # RoPE tables, extended to the 128-partition tile layout
    inv_freq = (1.0 / (THETA ** (np.arange(0, HD, 2, dtype=np.float32) / HD))).astype(
        np.float32
    )
    freqs = pos[:, None] * inv_freq[None, :]  # (S, 32)
    cosT = np.cos(freqs).astype(np.float32).T  # (32, S)
    sinT = np.sin(freqs).astype(np.float32).T
    cos_l = np.ascontiguousarray(np.tile(cosT, (4, 1)))  # (128, S)
    sin_l = np.ascontiguousarray(
        np.concatenate([-sinT, sinT, -sinT, sinT], axis=0)
    )

    # 128x128 half-swap permutation (block diag of two 64-blocks)
    p64 = np.zeros((64, 64), np.float32)
    for i in range(64):
        p64[i, (i + 32) % 64] = 1.0
    perm_l = np.zeros((128, 128), np.float32)
    perm_l[:64, :64] = p64
    perm_l[64:, 64:] = p64

    tri_l = (np.arange(128)[None, :] >= np.arange(128)[:, None]).astype(np.float32)

    in_maps = []
    for c in range(8):
        b, hg = divmod(c, 4)
        rows = hg * NCH + np.arange(NCH)
        # per-head deinterleave for q/k channel rows
        rows_de = (rows.reshape(HPC, HD)[:, de]).reshape(-1)

        xT = np.ascontiguousarray(x[b].T)  # (D, S)
        wq_t = np.ascontiguousarray(wq[rows_de, :].T)  # (D, 256)
        wk_t = np.ascontiguousarray(wk[rows_de, :].T)
        wv_t = np.ascontiguousarray(wv[rows, :].T)
        wo_t = np.ascontiguousarray(wo[:, rows].T)  # (256, D)

        in_maps.append(
            {
                "xT": _sbuf_layout(xT).astype(BF16),
                "wq": _sbuf_layout(wq_t).astype(BF16),
                "wk": _sbuf_layout(wk_t).astype(BF16),
                "wv": _sbuf_layout(wv_t).astype(BF16),
                "wo": _sbuf_layout(wo_t).astype(BF16),
                "cosd": cos_l,
                "sind": sin_l,
                "perm": perm_l.astype(BF16),
                "tri": tri_l.astype(BF16),
            }
        )
    return in_maps


def _get_nc():
    if "nc" not in _CACHE:
        _CACHE["nc"] = _build_nc()
    return _CACHE["nc"]


def kernel(x, wq, wk, wv, wo, token_positions, _trace=False, _tmpdir=None):
    nc = _get_nc()
    in_maps = _host_prep(x, wq, wk, wv, wo, token_positions)
    res = run_bass_kernel_spmd(
        nc, in_maps, core_ids=list(range(8)), trace=_trace, tmpdir=_tmpdir
    )
    out = np.zeros((B, S, D), np.float32)
    for c in range(8):
        b = c // 4
        out[b] += res.results[c]["o"]
    if _trace:
        kernel._last_result = res
    return out
